# revision 9
# baseline (speedup 1.0000x reference)
"""Trainium2 Bass kernel for nn_AdaLNConditioning (HGRNBitMLP + AdaLN head).

Strategy:
- Data-parallel over tokens: 8192 tokens -> 1024 per core, no collectives.
- Host precomputes ternary weight quantization (BitNet b1.58 global-mean
  scale) and packs transposed weight tiles in streaming order as bf16.
- On device, per token tile [128, D]: RMSNorm stats + per-token int8
  quantization (round-to-nearest-even via the 1.5*2^23 magic constant,
  bit-exact with jnp.round), quantized codes stored as bf16 (integers
  <= 127 are exact in bf16), PE-transposed into [K, token] layout.
- Matmuls run in bf16 on integer codes with f32 PSUM accumulation ->
  exact integer arithmetic; per-token dequant scale applied at PSUM
  evacuation (fused into ScalarE/VectorE copy).
- swiglu intermediate z and down-proj output h round-trip through HBM
  in f32 (bf16 storage costs ~1.5e-2 rel err; f32 keeps e2e ~1.3e-3).
"""

import sys
from contextlib import ExitStack

import numpy as np
import ml_dtypes

sys.path.insert(0, "/opt/trn_rl_repo")

import concourse.bass as bass  # noqa: E402
import concourse.tile as tile  # noqa: E402
from concourse import bacc  # noqa: E402
from concourse import mybir  # noqa: E402
from concourse.masks import make_identity  # noqa: E402

AF = mybir.ActivationFunctionType
ALU = mybir.AluOpType
F32 = mybir.dt.float32
BF16 = mybir.dt.bfloat16

P = 128
MAGIC = 12582912.0  # 1.5 * 2**23: add+store rounds f32 to nearest-even integer
N_CORES = 8


class Cfg:
    def __init__(self, T=1024, D=4096, INTER=4096, CW=512, KB=4):
        self.T = T            # tokens per core
        self.D = D            # model dim (k of L1/L3, out of L2/L3)
        self.INTER = INTER    # swiglu intermediate
        self.CW = CW          # output-chunk width (matmul moving free dim)
        self.KB = KB          # k-tiles per weight DMA batch
        self.TT = T // P
        self.GCH = 2 * INTER // CW  # L1 chunks (v/gate interleaved)
        self.DCH = D // CW          # L2/L3 chunks
        self.KT1 = D // P
        self.KT2 = INTER // P


def host_weight_quant(w):
    """BitNet ternary quant. Returns (codes {-1,0,1} f32, scale) matching
    jnp: scale = 1/clip(mean|w|, 1e-5); q = clip(round(w*scale), -1, 1)."""
    mean_abs = np.mean(np.abs(w), dtype=np.float64).astype(np.float32)
    s = np.float32(1.0) / np.maximum(mean_abs, np.float32(1e-5))
    q = np.clip(np.round(w * s), -1, 1).astype(np.float32)
    return q, s


def pack_weight(WqT, col_starts, cfg):
    """Pack WqT [K, O] into [n_chunks, KG, P, KB, CW] bf16 streaming layout."""
    K = WqT.shape[0]
    KT = K // P
    KG = KT // cfg.KB
    out = np.empty((len(col_starts), KG, P, cfg.KB, cfg.CW), dtype=ml_dtypes.bfloat16)
    for ci, c0 in enumerate(col_starts):
        blk = WqT[:, c0:c0 + cfg.CW]                       # [K, CW]
        blk = blk.reshape(KG, cfg.KB, P, cfg.CW).transpose(0, 2, 1, 3)
        out[ci] = blk.astype(ml_dtypes.bfloat16)
    return out


def build_nc(cfg, sg, sd, so):
    """Build the single-core (SPMD) Bass program."""
    nc = bacc.Bacc()
    T, D, INTER, CW, KB, TT = cfg.T, cfg.D, cfg.INTER, cfg.CW, cfg.KB, cfg.TT
    KT1, KT2, GCH, DCH = cfg.KT1, cfg.KT2, cfg.GCH, cfg.DCH
    KG1, KG2 = KT1 // KB, KT2 // KB
    TH = max(1, TT // 2)          # token tiles per evac half
    NH = (TT + TH - 1) // TH      # evac halves (2)
    QW = min(1024, D)             # quant sub-chunk width

    x_p = nc.declare_dram_parameter("x", [T, D], F32, isOutput=False)
    wg_p = nc.declare_dram_parameter("wg", [GCH, KG1, P, KB, CW], BF16, isOutput=False)
    wd_p = nc.declare_dram_parameter("wd", [DCH, KG2, P, KB, CW], BF16, isOutput=False)
    wo_p = nc.declare_dram_parameter("wo", [DCH, KG1, P, KB, CW], BF16, isOutput=False)
    nw_p = nc.declare_dram_parameter("nw", [1, D], F32, isOutput=False)
    out_p = nc.declare_dram_parameter("out", [T, D], F32, isOutput=True)

    c_gate = float(1.0 / (127.0 * sg))
    c_down = float(1.0 / (127.0 * sd))
    c_out = float(1.0 / (127.0 * so))

    with ExitStack() as ctx:
        tc = ctx.enter_context(tile.TileContext(nc))
        singles = ctx.enter_context(tc.tile_pool(name="singles", bufs=1))
        small = ctx.enter_context(tc.tile_pool(name="small", bufs=48))
        xin = ctx.enter_context(tc.tile_pool(name="xin", bufs=2))      # [P,D] f32
        rts = ctx.enter_context(tc.tile_pool(name="rts", bufs=3))      # [P,QW] f32 scratch
        qpool = ctx.enter_context(tc.tile_pool(name="qpool", bufs=2))  # [P,D] bf16
        qt_pool = ctx.enter_context(tc.tile_pool(name="qt", bufs=1))   # [P,KT,T] bf16
        wpool = ctx.enter_context(tc.tile_pool(name="wpool", bufs=3))  # [P,KB,CW] bf16
        gv = ctx.enter_context(tc.tile_pool(name="gv", bufs=2))        # [P,TH,CW] f32 per tag
        zpool = ctx.enter_context(tc.tile_pool(name="zpool", bufs=2))  # [P,TH,CW] f32
        psum = ctx.enter_context(tc.tile_pool(name="psum", bufs=8, space="PSUM"))
        dram = ctx.enter_context(tc.tile_pool(name="dram", bufs=1, space="DRAM"))

        ident = singles.tile([P, P], BF16)
        make_identity(nc, ident)
        eps_t = {}
        for ev in (1e-8, 1e-6):
            et = singles.tile([P, 1], F32, name=f"eps{ev}")
            nc.vector.memset(et, ev)
            eps_t[ev] = et
        nw_bc = singles.tile([P, D], F32)
        nw_ap = nw_p[:]
        nc.sync.dma_start(
            out=nw_bc,
            in_=bass.AP(tensor=nw_ap.tensor, offset=nw_ap.offset, ap=[[0, P], [1, D]]),
        )

        TPB = min(8, KT1)  # transposes batched per PSUM bank (8*128 bf16 = 2KB)

        def reduce_cols(parts, fn):
            """Combine [P,1] tiles with a binary DVE op; returns final tile."""
            while len(parts) > 1:
                nxt = []
                for i in range(0, len(parts) - 1, 2):
                    o = small.tile([P, 1], F32, tag="s", name="comb")
                    fn(o, parts[i], parts[i + 1])
                    nxt.append(o)
                if len(parts) % 2:
                    nxt.append(parts[-1])
                parts = nxt
            return parts[0]

        def quant_phase(src_ap, KTn, qTt, c_t, c_const, eps, nw=False, nw_eps=None):
            DL = KTn * P
            NQ = DL // QW
            for tt in range(TT):
                x_t = xin.tile([P, DL], F32, tag="xin")
                nc.sync.dma_start(out=x_t, in_=src_ap[tt * P:(tt + 1) * P, :])
                # sum of squares of x (sub-chunked; scratch slices from rts)
                sparts = []
                for j in range(NQ):
                    so_ = rts.tile([P, QW], F32, tag="rt")
                    ssj = small.tile([P, 1], F32, tag="s", name="ssj")
                    nc.scalar.activation(so_, x_t[:, j * QW:(j + 1) * QW], AF.Square,
                                         accum_out=ssj)
                    sparts.append(ssj)
                ssq = reduce_cols(sparts, nc.vector.tensor_add)
                r = small.tile([P, 1], F32, tag="s")
                if not nw:
                    std = small.tile([P, 1], F32, tag="s")
                    nc.scalar.activation(std, ssq, AF.Sqrt, scale=1.0 / DL, bias=eps_t[eps])
                    nc.vector.reciprocal(r, std)
                else:
                    # reference: h1 = h * rsqrt(mean h^2 + nw_eps) * nw, then
                    # bit_linear renorms: h2 = h1 * rsqrt(mean h1^2 + eps).
                    # Both fold into one per-token factor r = r1*r2 on (h*nw).
                    std1 = small.tile([P, 1], F32, tag="s")
                    nc.scalar.activation(std1, ssq, AF.Sqrt, scale=1.0 / DL,
                                         bias=eps_t[nw_eps])
                    r1 = small.tile([P, 1], F32, tag="s")
                    nc.vector.reciprocal(r1, std1)
                    s2parts = []
                    nw_aparts = []
                    for j in range(NQ):
                        t2j = rts.tile([P, QW], F32, tag="rt")
                        nc.vector.tensor_mul(t2j, x_t[:, j * QW:(j + 1) * QW],
                                             nw_bc[:, j * QW:(j + 1) * QW])
                        so2 = rts.tile([P, QW], F32, tag="rt")
                        ss2j = small.tile([P, 1], F32, tag="s", name="ss2j")
                        nc.scalar.activation(so2, t2j, AF.Square, accum_out=ss2j)
                        s2parts.append(ss2j)
                        amj = small.tile([P, 1], F32, tag="s", name="amj")
                        nc.vector.tensor_reduce(amj, t2j, axis=mybir.AxisListType.X,
                                                op=ALU.max, apply_absolute_value=True)
                        nw_aparts.append(amj)
                    ssq2 = reduce_cols(s2parts, nc.vector.tensor_add)
                    u = small.tile([P, 1], F32, tag="s")
                    nc.vector.tensor_mul(u, r1, r1)
                    w2 = small.tile([P, 1], F32, tag="s")
                    nc.vector.tensor_mul(w2, u, ssq2)
                    std2 = small.tile([P, 1], F32, tag="s")
                    nc.scalar.activation(std2, w2, AF.Sqrt, scale=1.0 / DL, bias=eps_t[eps])
                    r2 = small.tile([P, 1], F32, tag="s")
                    nc.vector.reciprocal(r2, std2)
                    nc.vector.tensor_mul(r, r1, r2)
                # abs-max of t2 (x or x*nw)
                if not nw:
                    aparts = []
                    for j in range(NQ):
                        amj = small.tile([P, 1], F32, tag="s", name="amj")
                        nc.vector.tensor_reduce(amj, x_t[:, j * QW:(j + 1) * QW],
                                                axis=mybir.AxisListType.X, op=ALU.max,
                                                apply_absolute_value=True)
                        aparts.append(amj)
                else:
                    aparts = nw_aparts
                am = reduce_cols(aparts, nc.vector.tensor_max)
                m0 = small.tile([P, 1], F32, tag="s")
                nc.vector.tensor_mul(m0, am, r)
                m = small.tile([P, 1], F32, tag="s")
                nc.vector.tensor_scalar_max(m, m0, 1e-5)
                invm = small.tile([P, 1], F32, tag="s")
                nc.vector.reciprocal(invm, m)
                qs = small.tile([P, 1], F32, tag="s")
                nc.vector.tensor_scalar(qs, r, scalar1=invm, scalar2=127.0,
                                        op0=ALU.mult, op1=ALU.mult)
                nc.vector.tensor_scalar(c_t[:, tt:tt + 1], m, scalar1=c_const,
                                        scalar2=None, op0=ALU.mult)
                # quantize: q = (t2*qs + MAGIC) - MAGIC, output bf16 (exact ints)
                q_t = qpool.tile([P, DL], BF16, tag="q")
                for j in range(NQ):
                    sl = slice(j * QW, (j + 1) * QW)
                    if not nw:
                        src_j = x_t[:, sl]
                    else:
                        src_j = rts.tile([P, QW], F32, tag="rt")
                        nc.vector.tensor_mul(src_j, x_t[:, sl], nw_bc[:, sl])
                    rt_j = rts.tile([P, QW], F32, tag="rt")
                    nc.scalar.activation(rt_j, src_j, AF.Copy, bias=MAGIC, scale=qs)
                    nc.vector.tensor_scalar_add(q_t[:, sl], rt_j, -MAGIC)
                # transpose q into [k, token] layout via PE
                for g in range(KTn // TPB):
                    ps = psum.tile([P, TPB * P], BF16, tag="ps")
                    for j in range(TPB):
                        kt = g * TPB + j
                        nc.tensor.transpose(ps[:, j * P:(j + 1) * P],
                                            q_t[:, kt * P:(kt + 1) * P], ident)
                    nc.vector.tensor_copy(
                        qTt[:, g * TPB:(g + 1) * TPB, tt * P:(tt + 1) * P],
                        ps.rearrange("p (j c) -> p j c", j=TPB),
                    )

        def mm_phase(w_p, nch, KTn, qTt, evac):
            # matmuls run in NH half-passes of TH token tiles (4 PSUM banks
            # each) so one half's evac overlaps the other half's matmuls
            KGn = KTn // KB
            for c in range(nch):
                wts = []
                for g in range(KGn):
                    wt = wpool.tile([P, KB, CW], BF16, tag="w")
                    nc.sync.dma_start(out=wt, in_=w_p[c, g])
                    wts.append(wt)
                for h in range(NH):
                    pss = []
                    for i in range(TH):
                        ps = psum.tile([P, CW], F32, tag="ps")
                        pss.append(ps)
                    for kt in range(KTn):
                        rhs = wts[kt // KB][:, kt % KB, :]
                        for i in range(TH):
                            tt = h * TH + i
                            nc.tensor.matmul(
                                pss[i],
                                lhsT=qTt[:, kt, tt * P:(tt + 1) * P],
                                rhs=rhs,
                                start=(kt == 0),
                                stop=(kt == KTn - 1),
                            )
                    evac(c, h, pss)

        # ---- Layer 1: gate_proj + swiglu -> z (HBM f32) ----
        c1 = singles.tile([P, TT], F32)
        qT1 = qt_pool.tile([P, KT1, T], BF16, tag="qt")
        quant_phase(x_p[:], KT1, qT1, c1, c_gate, 1e-8)

        z_d = dram.tile([T, INTER], F32, tag="zh")
        z_r = z_d[:].rearrange("(a p) n -> p a n", p=P)
        st = {}

        def evac1(c, h, pss):
            # chunk order is v0,g0,v1,g1,...: v is dequantized (x c1^2, the
            # extra c1 pre-applies gate's dequant), gate stays raw in PSUM;
            # z = (psum_g * sigmoid(psum_g*c1)) * v
            gi = c // 2
            if c % 2 == 0:
                v_t = gv.tile([P, TH, CW], F32, tag="v")
                for i in range(TH):
                    tt = h * TH + i
                    cc = c1[:, tt:tt + 1]
                    nc.vector.tensor_scalar(v_t[:, i, :], pss[i], scalar1=cc,
                                            scalar2=cc, op0=ALU.mult, op1=ALU.mult)
                st[("v", h)] = v_t
            else:
                v_t = st[("v", h)]
                sig_t = gv.tile([P, TH, CW], F32, tag="sig")
                z_t = zpool.tile([P, TH, CW], F32, tag="z")
                for i in range(TH):
                    tt = h * TH + i
                    cc = c1[:, tt:tt + 1]
                    nc.scalar.activation(sig_t[:, i, :], pss[i], AF.Sigmoid, scale=cc)
                    nc.vector.tensor_mul(z_t[:, i, :], pss[i], sig_t[:, i, :])
                nc.vector.tensor_mul(z_t, z_t, v_t)
                nc.sync.dma_start(
                    out=z_r[:, h * TH:(h + 1) * TH, gi * CW:(gi + 1) * CW], in_=z_t
                )

        mm_phase(wg_p, GCH, KT1, qT1, evac1)

        # ---- Layer 2: down_proj -> h (HBM f32) ----
        c2 = singles.tile([P, TT], F32)
        qT2 = qt_pool.tile([P, KT2, T], BF16, tag="qt")
        quant_phase(z_d[:], KT2, qT2, c2, c_down, 1e-8)

        h_d = dram.tile([T, D], F32, tag="zh")
        h_r = h_d[:].rearrange("(a p) n -> p a n", p=P)

        def make_evac_plain(c_t, dst_r):
            def evac(c, h, pss):
                o_t = zpool.tile([P, TH, CW], F32, tag="z")
                for i in range(TH):
                    tt = h * TH + i
                    cc = c_t[:, tt:tt + 1]
                    if tt % 2 == 0:
                        nc.vector.tensor_scalar(o_t[:, i, :], pss[i], scalar1=cc,
                                                scalar2=None, op0=ALU.mult)
                    else:
                        nc.scalar.activation(o_t[:, i, :], pss[i], AF.Copy, scale=cc)
                nc.sync.dma_start(
                    out=dst_r[:, h * TH:(h + 1) * TH, c * CW:(c + 1) * CW], in_=o_t
                )
            return evac

        mm_phase(wd_p, DCH, KT2, qT2, make_evac_plain(c2, h_r))

        # ---- Layer 3: AdaLN head: weighted RMSNorm + out_proj ----
        c3 = singles.tile([P, TT], F32)
        qT3 = qt_pool.tile([P, KT1, T], BF16, tag="qt")
        quant_phase(h_d[:], KT1, qT3, c3, c_out, 1e-8, nw=True, nw_eps=1e-6)

        out_r = out_p[:].rearrange("(a p) n -> p a n", p=P)
        mm_phase(wo_p, DCH, KT1, qT3, make_evac_plain(c3, out_r))

    return nc


def prepare_inputs(condition, w_gate, w_down, norm_weight, w_out, cfg, n_cores=N_CORES):
    """Host-side: quantize+pack weights, shard tokens. Returns (in_maps, scales)."""
    TOK = condition.shape[0] * condition.shape[1]
    X = np.ascontiguousarray(condition.reshape(TOK, cfg.D).astype(np.float32, copy=False))

    Wg, sg = host_weight_quant(np.asarray(w_gate, dtype=np.float32))
    Wd, sd = host_weight_quant(np.asarray(w_down, dtype=np.float32))
    Wo, so = host_weight_quant(np.asarray(w_out, dtype=np.float32))

    # L1 chunk order interleaves v/gate so swiglu can fuse per chunk pair
    l1_cols = []
    for i in range(cfg.INTER // cfg.CW):
        l1_cols += [cfg.INTER + i * cfg.CW, i * cfg.CW]
    WG = pack_weight(Wg.T, l1_cols, cfg)
    WD = pack_weight(Wd.T, [i * cfg.CW for i in range(cfg.D // cfg.CW)], cfg)
    WO = pack_weight(Wo.T, [i * cfg.CW for i in range(cfg.D // cfg.CW)], cfg)

    nw = np.ascontiguousarray(np.asarray(norm_weight, dtype=np.float32).reshape(1, cfg.D))

    in_maps = []
    for i in range(n_cores):
        in_maps.append({
            "x": np.ascontiguousarray(X[i * cfg.T:(i + 1) * cfg.T]),
            "wg": WG, "wd": WD, "wo": WO, "nw": nw,
        })
    return in_maps, (sg, sd, so)


def run(condition, w_gate, w_down, norm_weight, w_out, cfg=None, trace=False, tmpdir=None):
    from concourse.bass_utils import run_bass_kernel_spmd
    if cfg is None:
        cfg = Cfg()
    in_maps, (sg, sd, so) = prepare_inputs(condition, w_gate, w_down, norm_weight, w_out, cfg)
    nc = build_nc(cfg, sg, sd, so)
    nc.finalize()
    res = run_bass_kernel_spmd(nc, in_maps, list(range(N_CORES)), trace=trace, tmpdir=tmpdir)
    outs = np.concatenate([np.asarray(res.results[i]["out"]) for i in range(N_CORES)], axis=0)
    B, S = condition.shape[0], condition.shape[1]
    Pfull = outs.reshape(B, S, cfg.D)
    H = cfg.D // 2
    return (Pfull[..., :H], Pfull[..., H:]), res


def kernel(condition, w_gate, w_down, norm_weight, w_out):
    (scale, shift), _ = run(condition, w_gate, w_down, norm_weight, w_out)
    return scale, shift


# revision 11
# speedup vs baseline: 1.0626x; 1.0626x over previous
"""Trainium2 Bass kernel for nn_AdaLNConditioning (HGRNBitMLP + AdaLN head).

Strategy:
- Data-parallel over tokens: 8192 tokens -> 1024 per core, no collectives.
- Host precomputes ternary weight quantization (BitNet b1.58 global-mean
  scale) and packs transposed weight tiles in streaming order as bf16.
- On device, per token tile [128, D]: RMSNorm stats + per-token int8
  quantization (round-to-nearest-even via the 1.5*2^23 magic constant,
  bit-exact with jnp.round), quantized codes stored as bf16 (integers
  <= 127 are exact in bf16), PE-transposed into [K, token] layout.
- Matmuls run in bf16 on integer codes with f32 PSUM accumulation ->
  exact integer arithmetic; per-token dequant scale applied at PSUM
  evacuation (fused into ScalarE/VectorE copy).
- swiglu intermediate z and down-proj output h round-trip through HBM
  in f32 (bf16 storage costs ~1.5e-2 rel err; f32 keeps e2e ~1.3e-3).
"""

import sys
from contextlib import ExitStack

import numpy as np
import ml_dtypes

sys.path.insert(0, "/opt/trn_rl_repo")

import concourse.bass as bass  # noqa: E402
import concourse.tile as tile  # noqa: E402
from concourse import bacc  # noqa: E402
from concourse import mybir  # noqa: E402
from concourse.masks import make_identity  # noqa: E402

AF = mybir.ActivationFunctionType
ALU = mybir.AluOpType
F32 = mybir.dt.float32
BF16 = mybir.dt.bfloat16

P = 128
MAGIC = 12582912.0  # 1.5 * 2**23: add+store rounds f32 to nearest-even integer
N_CORES = 8


class Cfg:
    def __init__(self, T=1024, D=4096, INTER=4096, CW=512, KB=4):
        self.T = T            # tokens per core
        self.D = D            # model dim (k of L1/L3, out of L2/L3)
        self.INTER = INTER    # swiglu intermediate
        self.CW = CW          # output-chunk width (matmul moving free dim)
        self.KB = KB          # k-tiles per weight DMA batch
        self.TT = T // P
        self.GCH = 2 * INTER // CW  # L1 chunks (v/gate interleaved)
        self.DCH = D // CW          # L2/L3 chunks
        self.KT1 = D // P
        self.KT2 = INTER // P


def host_weight_quant(w):
    """BitNet ternary quant. Returns (codes {-1,0,1} f32, scale) matching
    jnp: scale = 1/clip(mean|w|, 1e-5); q = clip(round(w*scale), -1, 1)."""
    mean_abs = np.mean(np.abs(w), dtype=np.float64).astype(np.float32)
    s = np.float32(1.0) / np.maximum(mean_abs, np.float32(1e-5))
    q = np.clip(np.round(w * s), -1, 1).astype(np.float32)
    return q, s


def pack_weight(WqT, col_starts, cfg):
    """Pack WqT [K, O] into [n_chunks, KG, P, KB, CW] bf16 streaming layout."""
    K = WqT.shape[0]
    KT = K // P
    KG = KT // cfg.KB
    out = np.empty((len(col_starts), KG, P, cfg.KB, cfg.CW), dtype=ml_dtypes.bfloat16)
    for ci, c0 in enumerate(col_starts):
        blk = WqT[:, c0:c0 + cfg.CW]                       # [K, CW]
        blk = blk.reshape(KG, cfg.KB, P, cfg.CW).transpose(0, 2, 1, 3)
        out[ci] = blk.astype(ml_dtypes.bfloat16)
    return out


def build_nc(cfg, sg, sd, so):
    """Build the single-core (SPMD) Bass program."""
    nc = bacc.Bacc()
    T, D, INTER, CW, KB, TT = cfg.T, cfg.D, cfg.INTER, cfg.CW, cfg.KB, cfg.TT
    KT1, KT2, GCH, DCH = cfg.KT1, cfg.KT2, cfg.GCH, cfg.DCH
    KG1, KG2 = KT1 // KB, KT2 // KB
    TH = max(1, TT // 2)          # token tiles per evac half
    NH = (TT + TH - 1) // TH      # evac halves (2)
    QW = min(1024, D)             # quant sub-chunk width

    x_p = nc.declare_dram_parameter("x", [T, D], F32, isOutput=False)
    wg_p = nc.declare_dram_parameter("wg", [GCH, KG1, P, KB, CW], BF16, isOutput=False)
    wd_p = nc.declare_dram_parameter("wd", [DCH, KG2, P, KB, CW], BF16, isOutput=False)
    wo_p = nc.declare_dram_parameter("wo", [DCH, KG1, P, KB, CW], BF16, isOutput=False)
    nw_p = nc.declare_dram_parameter("nw", [1, D], F32, isOutput=False)
    out_p = nc.declare_dram_parameter("out", [T, D], F32, isOutput=True)

    c_gate = float(1.0 / (127.0 * sg))
    c_down = float(1.0 / (127.0 * sd))
    c_out = float(1.0 / (127.0 * so))

    with ExitStack() as ctx:
        tc = ctx.enter_context(tile.TileContext(nc))
        singles = ctx.enter_context(tc.tile_pool(name="singles", bufs=1))
        small = ctx.enter_context(tc.tile_pool(name="small", bufs=48))
        xin = ctx.enter_context(tc.tile_pool(name="xin", bufs=2))      # [P,D] f32
        rts = ctx.enter_context(tc.tile_pool(name="rts", bufs=4))      # [P,QW] f32 scratch
        qt_pool = ctx.enter_context(tc.tile_pool(name="qt", bufs=2))   # [P,KT,TH*P] bf16
        wpool = ctx.enter_context(tc.tile_pool(name="wpool", bufs=3))  # [P,KB,CW] bf16
        gv = ctx.enter_context(tc.tile_pool(name="gv", bufs=2))        # [P,TH,CW] f32 per tag
        zpool = ctx.enter_context(tc.tile_pool(name="zpool", bufs=2))  # [P,TH,CW] f32
        psum = ctx.enter_context(tc.tile_pool(name="psum", bufs=8, space="PSUM"))
        dram = ctx.enter_context(tc.tile_pool(name="dram", bufs=1, space="DRAM"))

        identf = singles.tile([P, P], F32)
        make_identity(nc, identf)
        eps_t = {}
        for ev in (1e-8, 1e-6):
            et = singles.tile([P, 1], F32, name=f"eps{ev}")
            nc.vector.memset(et, ev)
            eps_t[ev] = et
        nw_bc = singles.tile([P, D], F32)
        nw_ap = nw_p[:]
        nc.sync.dma_start(
            out=nw_bc,
            in_=bass.AP(tensor=nw_ap.tensor, offset=nw_ap.offset, ap=[[0, P], [1, D]]),
        )

        TPB = min(4, KT1)  # f32 transposes batched per PSUM bank (4*128*4B = 2KB)

        def reduce_cols(parts, fn):
            """Combine [P,1] tiles with a binary DVE op; returns final tile."""
            while len(parts) > 1:
                nxt = []
                for i in range(0, len(parts) - 1, 2):
                    o = small.tile([P, 1], F32, tag="s", name="comb")
                    fn(o, parts[i], parts[i + 1])
                    nxt.append(o)
                if len(parts) % 2:
                    nxt.append(parts[-1])
                parts = nxt
            return parts[0]

        def quant_half(src_ap, KTn, h, qTt, c_t, c_const, eps, nw=False, nw_eps=None):
            """Norm + int8-quant + transpose for token tiles of half h.

            Quant scale is qs = 127/absmax(t2) (the rsqrt factor cancels
            algebraically between quant and dequant); the dequant scale
            c = (am*c_const)*r carries the norm factor r off the critical
            path. rt = (t2*qs + MAGIC) rounds to integer+MAGIC at the f32
            store; the -MAGIC subtract is folded into the PSUM->SBUF
            transpose evacuation (bf16 output, exact for ints <= 127).
            """
            DL = KTn * P
            NQ = DL // QW
            for i in range(TH):
                tt = h * TH + i
                x_t = xin.tile([P, DL], F32, tag="xin")
                nc.sync.dma_start(out=x_t, in_=src_ap[tt * P:(tt + 1) * P, :])
                sparts = []
                for j in range(NQ):
                    so_ = rts.tile([P, QW], F32, tag="rt")
                    ssj = small.tile([P, 1], F32, tag="s", name="ssj")
                    nc.scalar.activation(so_, x_t[:, j * QW:(j + 1) * QW], AF.Square,
                                         accum_out=ssj)
                    sparts.append(ssj)
                ssq = reduce_cols(sparts, nc.vector.tensor_add)
                r = small.tile([P, 1], F32, tag="s")
                aparts = []
                if not nw:
                    std = small.tile([P, 1], F32, tag="s")
                    nc.scalar.activation(std, ssq, AF.Sqrt, scale=1.0 / DL, bias=eps_t[eps])
                    nc.vector.reciprocal(r, std)
                    for j in range(NQ):
                        amj = small.tile([P, 1], F32, tag="s", name="amj")
                        nc.vector.tensor_reduce(amj, x_t[:, j * QW:(j + 1) * QW],
                                                axis=mybir.AxisListType.X, op=ALU.max,
                                                apply_absolute_value=True)
                        aparts.append(amj)
                else:
                    # reference: h1 = h * rsqrt(mean h^2 + nw_eps) * nw, then
                    # bit_linear renorms: h2 = h1 * rsqrt(mean h1^2 + eps).
                    # Both fold into one per-token factor r = r1*r2 on (h*nw).
                    std1 = small.tile([P, 1], F32, tag="s")
                    nc.scalar.activation(std1, ssq, AF.Sqrt, scale=1.0 / DL,
                                         bias=eps_t[nw_eps])
                    r1 = small.tile([P, 1], F32, tag="s")
                    nc.vector.reciprocal(r1, std1)
                    s2parts = []
                    for j in range(NQ):
                        t2j = rts.tile([P, QW], F32, tag="t2", bufs=2)
                        nc.vector.tensor_mul(t2j, x_t[:, j * QW:(j + 1) * QW],
                                             nw_bc[:, j * QW:(j + 1) * QW])
                        so2 = rts.tile([P, QW], F32, tag="rt")
                        ss2j = small.tile([P, 1], F32, tag="s", name="ss2j")
                        nc.scalar.activation(so2, t2j, AF.Square, accum_out=ss2j)
                        s2parts.append(ss2j)
                        amj = small.tile([P, 1], F32, tag="s", name="amj")
                        nc.vector.tensor_reduce(amj, t2j, axis=mybir.AxisListType.X,
                                                op=ALU.max, apply_absolute_value=True)
                        aparts.append(amj)
                    ssq2 = reduce_cols(s2parts, nc.vector.tensor_add)
                    u = small.tile([P, 1], F32, tag="s")
                    nc.vector.tensor_mul(u, r1, r1)
                    w2 = small.tile([P, 1], F32, tag="s")
                    nc.vector.tensor_mul(w2, u, ssq2)
                    std2 = small.tile([P, 1], F32, tag="s")
                    nc.scalar.activation(std2, w2, AF.Sqrt, scale=1.0 / DL, bias=eps_t[eps])
                    r2 = small.tile([P, 1], F32, tag="s")
                    nc.vector.reciprocal(r2, std2)
                    nc.vector.tensor_mul(r, r1, r2)
                am = reduce_cols(aparts, nc.vector.tensor_max)
                invam = small.tile([P, 1], F32, tag="s")
                nc.vector.reciprocal(invam, am)
                qs = small.tile([P, 1], F32, tag="s")
                nc.vector.tensor_scalar_mul(qs, invam, 127.0)
                nc.vector.scalar_tensor_tensor(c_t[:, i:i + 1], am, c_const, r,
                                               op0=ALU.mult, op1=ALU.mult)
                # rt = t2*qs + MAGIC (f32 store rounds to nearest-even int)
                rtjs = []
                for j in range(NQ):
                    if not nw:
                        src_j = x_t[:, j * QW:(j + 1) * QW]
                    else:
                        src_j = rts.tile([P, QW], F32, tag="t2", bufs=2)
                        nc.vector.tensor_mul(src_j, x_t[:, j * QW:(j + 1) * QW],
                                             nw_bc[:, j * QW:(j + 1) * QW])
                    rt_j = rts.tile([P, QW], F32, tag="rt")
                    nc.vector.tensor_scalar(rt_j, src_j, scalar1=qs, scalar2=MAGIC,
                                            op0=ALU.mult, op1=ALU.add)
                    rtjs.append(rt_j)
                # PE-transpose rt (f32), subtract MAGIC on evac -> bf16 codes
                KTQ = QW // P
                for g in range(KTn // TPB):
                    ps = psum.tile([P, TPB * P], F32, tag="ps")
                    for j in range(TPB):
                        kt = g * TPB + j
                        rt_j = rtjs[kt // KTQ]
                        col = (kt % KTQ) * P
                        nc.tensor.transpose(ps[:, j * P:(j + 1) * P],
                                            rt_j[:, col:col + P], identf)
                    dst = qTt[:, g * TPB:(g + 1) * TPB, i * P:(i + 1) * P]
                    src = ps.rearrange("p (j c) -> p j c", j=TPB)
                    if g % 2 == 0:
                        nc.vector.tensor_scalar_add(dst, src, -MAGIC)
                    else:
                        nc.scalar.activation(dst, src, AF.Copy, bias=-MAGIC)

        def mm_half(w_p, nch, KTn, h, qTt, evac):
            KGn = KTn // KB
            for c in range(nch):
                wts = []
                for g in range(KGn):
                    wt = wpool.tile([P, KB, CW], BF16, tag="w")
                    nc.sync.dma_start(out=wt, in_=w_p[c, g])
                    wts.append(wt)
                pss = []
                for i in range(TH):
                    ps = psum.tile([P, CW], F32, tag="ps")
                    pss.append(ps)
                for kt in range(KTn):
                    rhs = wts[kt // KB][:, kt % KB, :]
                    for i in range(TH):
                        nc.tensor.matmul(
                            pss[i],
                            lhsT=qTt[:, kt, i * P:(i + 1) * P],
                            rhs=rhs,
                            start=(kt == 0),
                            stop=(kt == KTn - 1),
                        )
                evac(c, h, pss)

        # ---- dram intermediates (per half: no false cross-half deps) ----
        z_ds = [dram.tile([T // NH, INTER], F32, name=f"z{h}", tag=f"z{h}")
                for h in range(NH)]
        z_rs = [zd[:].rearrange("(a p) n -> p a n", p=P) for zd in z_ds]
        h_ds = [dram.tile([T // NH, D], F32, name=f"h{h}", tag=f"h{h}")
                for h in range(NH)]
        h_rs = [hd[:].rearrange("(a p) n -> p a n", p=P) for hd in h_ds]
        out_r = out_p[:].rearrange("(a p) n -> p a n", p=P)
        st = {}

        def make_evac1(c1h):
            def evac1(c, h, pss):
                # chunk order v0,g0,v1,g1,...: v dequantized x c1^2 (extra c1
                # pre-applies gate dequant); z = (psum_g * sigmoid(psum_g*c1)) * v
                gi = c // 2
                if c % 2 == 0:
                    v_t = gv.tile([P, TH, CW], F32, tag="v")
                    for i in range(TH):
                        cc = c1h[:, i:i + 1]
                        nc.vector.tensor_scalar(v_t[:, i, :], pss[i], scalar1=cc,
                                                scalar2=cc, op0=ALU.mult, op1=ALU.mult)
                    st["v"] = v_t
                else:
                    v_t = st["v"]
                    sig_t = gv.tile([P, TH, CW], F32, tag="sig")
                    z_t = zpool.tile([P, TH, CW], F32, tag="z")
                    for i in range(TH):
                        cc = c1h[:, i:i + 1]
                        nc.scalar.activation(sig_t[:, i, :], pss[i], AF.Sigmoid, scale=cc)
                        nc.vector.tensor_mul(z_t[:, i, :], pss[i], sig_t[:, i, :])
                    nc.vector.tensor_mul(z_t, z_t, v_t)
                    nc.sync.dma_start(
                        out=z_rs[h][:, :, gi * CW:(gi + 1) * CW], in_=z_t
                    )
            return evac1

        def make_evac_plain(c_th, dst_r, use_h_offset):
            def evac(c, h, pss):
                o_t = zpool.tile([P, TH, CW], F32, tag="z")
                for i in range(TH):
                    cc = c_th[:, i:i + 1]
                    if i % 2 == 0:
                        nc.vector.tensor_scalar(o_t[:, i, :], pss[i], scalar1=cc,
                                                scalar2=None, op0=ALU.mult)
                    else:
                        nc.scalar.activation(o_t[:, i, :], pss[i], AF.Copy, scale=cc)
                row0 = h * TH if use_h_offset else 0
                nc.sync.dma_start(
                    out=dst_r[:, row0:row0 + TH, c * CW:(c + 1) * CW], in_=o_t
                )
            return evac

        # ---- pipelined layers: quant(L, h+1) overlaps mm(L, h) ----
        for h in range(NH):
            c1h = singles.tile([P, TH], F32, name=f"c1_{h}")
            qT1 = qt_pool.tile([P, KT1, TH * P], BF16, tag="qt")
            quant_half(x_p[:], KT1, h, qT1, c1h, c_gate, 1e-8)
            mm_half(wg_p, GCH, KT1, h, qT1, make_evac1(c1h))

        for h in range(NH):
            c2h = singles.tile([P, TH], F32, name=f"c2_{h}")
            qT2 = qt_pool.tile([P, KT2, TH * P], BF16, tag="qt")
            quant_half(z_ds[h][:], KT2, 0, qT2, c2h, c_down, 1e-8)
            mm_half(wd_p, DCH, KT2, h, qT2, make_evac_plain(c2h, h_rs[h], False))

        for h in range(NH):
            c3h = singles.tile([P, TH], F32, name=f"c3_{h}")
            qT3 = qt_pool.tile([P, KT1, TH * P], BF16, tag="qt")
            quant_half(h_ds[h][:], KT1, 0, qT3, c3h, c_out, 1e-8, nw=True, nw_eps=1e-6)
            mm_half(wo_p, DCH, KT1, h, qT3, make_evac_plain(c3h, out_r, True))

    return nc


def prepare_inputs(condition, w_gate, w_down, norm_weight, w_out, cfg, n_cores=N_CORES):
    """Host-side: quantize+pack weights, shard tokens. Returns (in_maps, scales)."""
    TOK = condition.shape[0] * condition.shape[1]
    X = np.ascontiguousarray(condition.reshape(TOK, cfg.D).astype(np.float32, copy=False))

    Wg, sg = host_weight_quant(np.asarray(w_gate, dtype=np.float32))
    Wd, sd = host_weight_quant(np.asarray(w_down, dtype=np.float32))
    Wo, so = host_weight_quant(np.asarray(w_out, dtype=np.float32))

    # L1 chunk order interleaves v/gate so swiglu can fuse per chunk pair
    l1_cols = []
    for i in range(cfg.INTER // cfg.CW):
        l1_cols += [cfg.INTER + i * cfg.CW, i * cfg.CW]
    WG = pack_weight(Wg.T, l1_cols, cfg)
    WD = pack_weight(Wd.T, [i * cfg.CW for i in range(cfg.D // cfg.CW)], cfg)
    WO = pack_weight(Wo.T, [i * cfg.CW for i in range(cfg.D // cfg.CW)], cfg)

    nw = np.ascontiguousarray(np.asarray(norm_weight, dtype=np.float32).reshape(1, cfg.D))

    in_maps = []
    for i in range(n_cores):
        in_maps.append({
            "x": np.ascontiguousarray(X[i * cfg.T:(i + 1) * cfg.T]),
            "wg": WG, "wd": WD, "wo": WO, "nw": nw,
        })
    return in_maps, (sg, sd, so)


def run(condition, w_gate, w_down, norm_weight, w_out, cfg=None, trace=False, tmpdir=None):
    from concourse.bass_utils import run_bass_kernel_spmd
    if cfg is None:
        cfg = Cfg()
    in_maps, (sg, sd, so) = prepare_inputs(condition, w_gate, w_down, norm_weight, w_out, cfg)
    nc = build_nc(cfg, sg, sd, so)
    nc.finalize()
    res = run_bass_kernel_spmd(nc, in_maps, list(range(N_CORES)), trace=trace, tmpdir=tmpdir)
    outs = np.concatenate([np.asarray(res.results[i]["out"]) for i in range(N_CORES)], axis=0)
    B, S = condition.shape[0], condition.shape[1]
    Pfull = outs.reshape(B, S, cfg.D)
    H = cfg.D // 2
    return (Pfull[..., :H], Pfull[..., H:]), res


def kernel(condition, w_gate, w_down, norm_weight, w_out):
    (scale, shift), _ = run(condition, w_gate, w_down, norm_weight, w_out)
    return scale, shift


# revision 13
# speedup vs baseline: 1.1876x; 1.1176x over previous
"""Trainium2 Bass kernel for nn_AdaLNConditioning (HGRNBitMLP + AdaLN head).

Strategy:
- Data-parallel over tokens: 8192 tokens -> 1024 per core, no collectives.
- Host precomputes ternary weight quantization (BitNet b1.58 global-mean
  scale) and packs transposed weight tiles in streaming order as bf16.
- On device, per token tile [128, D]: RMSNorm stats + per-token int8
  quantization (round-to-nearest-even via the 1.5*2^23 magic constant,
  bit-exact with jnp.round), quantized codes stored as bf16 (integers
  <= 127 are exact in bf16), PE-transposed into [K, token] layout.
- Matmuls run in bf16 on integer codes with f32 PSUM accumulation ->
  exact integer arithmetic; per-token dequant scale applied at PSUM
  evacuation (fused into ScalarE/VectorE copy).
- swiglu intermediate z and down-proj output h round-trip through HBM
  in f32 (bf16 storage costs ~1.5e-2 rel err; f32 keeps e2e ~1.3e-3).
"""

import sys
from contextlib import ExitStack

import numpy as np
import ml_dtypes

sys.path.insert(0, "/opt/trn_rl_repo")

import concourse.bass as bass  # noqa: E402
import concourse.tile as tile  # noqa: E402
from concourse import bacc  # noqa: E402
from concourse import mybir  # noqa: E402
from concourse.masks import make_identity  # noqa: E402

AF = mybir.ActivationFunctionType
ALU = mybir.AluOpType
F32 = mybir.dt.float32
BF16 = mybir.dt.bfloat16

P = 128
MAGIC = 12582912.0  # 1.5 * 2**23: add+store rounds f32 to nearest-even integer
N_CORES = 8


class Cfg:
    def __init__(self, T=1024, D=4096, INTER=4096, CW=512, KB=4):
        self.T = T            # tokens per core
        self.D = D            # model dim (k of L1/L3, out of L2/L3)
        self.INTER = INTER    # swiglu intermediate
        self.CW = CW          # output-chunk width (matmul moving free dim)
        self.KB = KB          # k-tiles per weight DMA batch
        self.TT = T // P
        self.GCH = 2 * INTER // CW  # L1 chunks (v/gate interleaved)
        self.DCH = D // CW          # L2/L3 chunks
        self.KT1 = D // P
        self.KT2 = INTER // P


def host_weight_quant(w):
    """BitNet ternary quant. Returns (codes {-1,0,1} f32, scale) matching
    jnp: scale = 1/clip(mean|w|, 1e-5); q = clip(round(w*scale), -1, 1)."""
    mean_abs = np.mean(np.abs(w), dtype=np.float64).astype(np.float32)
    s = np.float32(1.0) / np.maximum(mean_abs, np.float32(1e-5))
    q = np.clip(np.round(w * s), -1, 1).astype(np.float32)
    return q, s


def pack_weight(WqT, col_starts, cfg):
    """Pack WqT [K, O] into [n_chunks, KG, P, KB, CW] bf16 streaming layout."""
    K = WqT.shape[0]
    KT = K // P
    KG = KT // cfg.KB
    out = np.empty((len(col_starts), KG, P, cfg.KB, cfg.CW), dtype=ml_dtypes.bfloat16)
    for ci, c0 in enumerate(col_starts):
        blk = WqT[:, c0:c0 + cfg.CW]                       # [K, CW]
        blk = blk.reshape(KG, cfg.KB, P, cfg.CW).transpose(0, 2, 1, 3)
        out[ci] = blk.astype(ml_dtypes.bfloat16)
    return out


def build_nc(cfg, sg, sd, so):
    """Build the single-core (SPMD) Bass program."""
    nc = bacc.Bacc()
    T, D, INTER, CW, KB, TT = cfg.T, cfg.D, cfg.INTER, cfg.CW, cfg.KB, cfg.TT
    KT1, KT2, GCH, DCH = cfg.KT1, cfg.KT2, cfg.GCH, cfg.DCH
    KG1, KG2 = KT1 // KB, KT2 // KB
    TH = max(1, TT // 2)          # token tiles per evac half
    NH = (TT + TH - 1) // TH      # evac halves (2)
    QW = min(1024, D)             # quant sub-chunk width

    x_p = nc.declare_dram_parameter("x", [T, D], F32, isOutput=False)
    wg_p = nc.declare_dram_parameter("wg", [GCH, KG1, P, KB, CW], BF16, isOutput=False)
    wd_p = nc.declare_dram_parameter("wd", [DCH, KG2, P, KB, CW], BF16, isOutput=False)
    wo_p = nc.declare_dram_parameter("wo", [DCH, KG1, P, KB, CW], BF16, isOutput=False)
    nw_p = nc.declare_dram_parameter("nw", [1, D], F32, isOutput=False)
    out_p = nc.declare_dram_parameter("out", [T, D], F32, isOutput=True)

    c_gate = float(1.0 / (127.0 * sg))
    c_down = float(1.0 / (127.0 * sd))
    c_out = float(1.0 / (127.0 * so))

    with ExitStack() as ctx:
        tc = ctx.enter_context(tile.TileContext(nc))
        singles = ctx.enter_context(tc.tile_pool(name="singles", bufs=1))
        small = ctx.enter_context(tc.tile_pool(name="small", bufs=48))
        xin = ctx.enter_context(tc.tile_pool(name="xin", bufs=2))      # [P,D] f32
        rts = ctx.enter_context(tc.tile_pool(name="rts", bufs=4))      # [P,QW] f32 scratch
        qt_pool = ctx.enter_context(tc.tile_pool(name="qt", bufs=2))   # [P,KT,TH*P] bf16
        wpool = ctx.enter_context(tc.tile_pool(name="wpool", bufs=3))  # [P,KB,CW] bf16
        gv = ctx.enter_context(tc.tile_pool(name="gv", bufs=2))        # [P,TH,CW] f32 per tag
        zpool = ctx.enter_context(tc.tile_pool(name="zpool", bufs=2))  # [P,TH,CW] f32
        mm_ps = ctx.enter_context(tc.tile_pool(name="mmps", bufs=6, space="PSUM"))
        tp_ps = ctx.enter_context(tc.tile_pool(name="tpps", bufs=2, space="PSUM"))
        dram = ctx.enter_context(tc.tile_pool(name="dram", bufs=1, space="DRAM"))

        identf = singles.tile([P, P], F32)
        make_identity(nc, identf)
        eps_t = {}
        for ev in (1e-8, 1e-6):
            et = singles.tile([P, 1], F32, name=f"eps{ev}")
            nc.vector.memset(et, ev)
            eps_t[ev] = et
        nw_bc = singles.tile([P, D], F32)
        nw_ap = nw_p[:]
        nc.sync.dma_start(
            out=nw_bc,
            in_=bass.AP(tensor=nw_ap.tensor, offset=nw_ap.offset, ap=[[0, P], [1, D]]),
        )

        TPB = min(4, KT1)  # f32 transposes batched per PSUM bank (4*128*4B = 2KB)

        def reduce_cols(parts, fn):
            """Combine [P,1] tiles with a binary DVE op; returns final tile."""
            while len(parts) > 1:
                nxt = []
                for i in range(0, len(parts) - 1, 2):
                    o = small.tile([P, 1], F32, tag="s", name="comb")
                    fn(o, parts[i], parts[i + 1])
                    nxt.append(o)
                if len(parts) % 2:
                    nxt.append(parts[-1])
                parts = nxt
            return parts[0]

        def quant_gen(src_ap, KTn, h, qTt, c_t, c_const, eps, nw=False, nw_eps=None):
            """Norm + int8-quant + transpose for token tiles of half h.

            Quant scale is qs = 127/absmax(t2) (the rsqrt factor cancels
            algebraically between quant and dequant); the dequant scale
            c = (am*c_const)*r carries the norm factor r off the critical
            path. rt = (t2*qs + MAGIC) rounds to integer+MAGIC at the f32
            store; the -MAGIC subtract is folded into the PSUM->SBUF
            transpose evacuation (bf16 output, exact for ints <= 127).
            """
            DL = KTn * P
            NQ = DL // QW
            for i in range(TH):
                tt = h * TH + i
                x_t = xin.tile([P, DL], F32, tag="xin")
                nc.sync.dma_start(out=x_t, in_=src_ap[tt * P:(tt + 1) * P, :])
                sparts = []
                for j in range(NQ):
                    so_ = rts.tile([P, QW], F32, tag="sq", bufs=2)
                    ssj = small.tile([P, 1], F32, tag="s", name="ssj")
                    nc.scalar.activation(so_, x_t[:, j * QW:(j + 1) * QW], AF.Square,
                                         accum_out=ssj)
                    sparts.append(ssj)
                ssq = reduce_cols(sparts, nc.vector.tensor_add)
                r = small.tile([P, 1], F32, tag="s")
                aparts = []
                if not nw:
                    std = small.tile([P, 1], F32, tag="s")
                    nc.scalar.activation(std, ssq, AF.Sqrt, scale=1.0 / DL, bias=eps_t[eps])
                    nc.vector.reciprocal(r, std)
                    for j in range(NQ):
                        amj = small.tile([P, 1], F32, tag="s", name="amj")
                        nc.vector.tensor_reduce(amj, x_t[:, j * QW:(j + 1) * QW],
                                                axis=mybir.AxisListType.X, op=ALU.max,
                                                apply_absolute_value=True)
                        aparts.append(amj)
                else:
                    # reference: h1 = h * rsqrt(mean h^2 + nw_eps) * nw, then
                    # bit_linear renorms: h2 = h1 * rsqrt(mean h1^2 + eps).
                    # Both fold into one per-token factor r = r1*r2 on (h*nw).
                    std1 = small.tile([P, 1], F32, tag="s")
                    nc.scalar.activation(std1, ssq, AF.Sqrt, scale=1.0 / DL,
                                         bias=eps_t[nw_eps])
                    r1 = small.tile([P, 1], F32, tag="s")
                    nc.vector.reciprocal(r1, std1)
                    s2parts = []
                    for j in range(NQ):
                        t2j = rts.tile([P, QW], F32, tag="t2", bufs=2)
                        nc.vector.tensor_mul(t2j, x_t[:, j * QW:(j + 1) * QW],
                                             nw_bc[:, j * QW:(j + 1) * QW])
                        so2 = rts.tile([P, QW], F32, tag="sq", bufs=2)
                        ss2j = small.tile([P, 1], F32, tag="s", name="ss2j")
                        nc.scalar.activation(so2, t2j, AF.Square, accum_out=ss2j)
                        s2parts.append(ss2j)
                        amj = small.tile([P, 1], F32, tag="s", name="amj")
                        nc.vector.tensor_reduce(amj, t2j, axis=mybir.AxisListType.X,
                                                op=ALU.max, apply_absolute_value=True)
                        aparts.append(amj)
                    ssq2 = reduce_cols(s2parts, nc.vector.tensor_add)
                    u = small.tile([P, 1], F32, tag="s")
                    nc.vector.tensor_mul(u, r1, r1)
                    w2 = small.tile([P, 1], F32, tag="s")
                    nc.vector.tensor_mul(w2, u, ssq2)
                    std2 = small.tile([P, 1], F32, tag="s")
                    nc.scalar.activation(std2, w2, AF.Sqrt, scale=1.0 / DL, bias=eps_t[eps])
                    r2 = small.tile([P, 1], F32, tag="s")
                    nc.vector.reciprocal(r2, std2)
                    nc.vector.tensor_mul(r, r1, r2)
                am = reduce_cols(aparts, nc.vector.tensor_max)
                invam = small.tile([P, 1], F32, tag="s")
                nc.vector.reciprocal(invam, am)
                qs = small.tile([P, 1], F32, tag="s")
                nc.vector.tensor_scalar_mul(qs, invam, 127.0)
                nc.vector.scalar_tensor_tensor(c_t[:, i:i + 1], am, c_const, r,
                                               op0=ALU.mult, op1=ALU.mult)
                # rt = t2*qs + MAGIC (f32 store rounds to nearest-even int)
                rtjs = []
                for j in range(NQ):
                    if not nw:
                        src_j = x_t[:, j * QW:(j + 1) * QW]
                    else:
                        src_j = rts.tile([P, QW], F32, tag="t2", bufs=2)
                        nc.vector.tensor_mul(src_j, x_t[:, j * QW:(j + 1) * QW],
                                             nw_bc[:, j * QW:(j + 1) * QW])
                    rt_j = rts.tile([P, QW], F32, tag="rt")
                    nc.vector.tensor_scalar(rt_j, src_j, scalar1=qs, scalar2=MAGIC,
                                            op0=ALU.mult, op1=ALU.add)
                    rtjs.append(rt_j)
                # PE-transpose rt (f32), subtract MAGIC on evac -> bf16 codes
                KTQ = QW // P
                for g in range(KTn // TPB):
                    ps = tp_ps.tile([P, TPB * P], F32, tag="tp")
                    for j in range(TPB):
                        kt = g * TPB + j
                        rt_j = rtjs[kt // KTQ]
                        col = (kt % KTQ) * P
                        nc.tensor.transpose(ps[:, j * P:(j + 1) * P],
                                            rt_j[:, col:col + P], identf)
                    dst = qTt[:, g * TPB:(g + 1) * TPB, i * P:(i + 1) * P]
                    src = ps.rearrange("p (j c) -> p j c", j=TPB)
                    if g % 2 == 0:
                        nc.vector.tensor_scalar_add(dst, src, -MAGIC)
                    else:
                        nc.scalar.activation(dst, src, AF.Copy, bias=-MAGIC)
                yield

        def mm_gen(w_p, nch, KTn, h, qTt, evac):
            KGn = KTn // KB
            for c in range(nch):
                wts = []
                for g in range(KGn):
                    wt = wpool.tile([P, KB, CW], BF16, tag="w")
                    nc.sync.dma_start(out=wt, in_=w_p[c, g])
                    wts.append(wt)
                pss = []
                for i in range(TH):
                    ps = mm_ps.tile([P, CW], F32, tag="mm")
                    pss.append(ps)
                for kt in range(KTn):
                    rhs = wts[kt // KB][:, kt % KB, :]
                    for i in range(TH):
                        nc.tensor.matmul(
                            pss[i],
                            lhsT=qTt[:, kt, i * P:(i + 1) * P],
                            rhs=rhs,
                            start=(kt == 0),
                            stop=(kt == KTn - 1),
                        )
                evac(c, h, pss)
                yield

        # ---- dram intermediates (per half: no false cross-half deps) ----
        z_ds = [dram.tile([T // NH, INTER], F32, name=f"z{h}", tag=f"z{h}")
                for h in range(NH)]
        z_rs = [zd[:].rearrange("(a p) n -> p a n", p=P) for zd in z_ds]
        h_ds = [dram.tile([T // NH, D], F32, name=f"h{h}", tag=f"h{h}")
                for h in range(NH)]
        h_rs = [hd[:].rearrange("(a p) n -> p a n", p=P) for hd in h_ds]
        out_r = out_p[:].rearrange("(a p) n -> p a n", p=P)
        st = {}

        def make_evac1(c1h):
            def evac1(c, h, pss):
                # chunk order v0,g0,v1,g1,...: v dequantized x c1^2 (extra c1
                # pre-applies gate dequant); z = (psum_g * sigmoid(psum_g*c1)) * v
                gi = c // 2
                if c % 2 == 0:
                    v_t = gv.tile([P, TH, CW], F32, tag="v")
                    for i in range(TH):
                        cc = c1h[:, i:i + 1]
                        nc.vector.tensor_scalar(v_t[:, i, :], pss[i], scalar1=cc,
                                                scalar2=cc, op0=ALU.mult, op1=ALU.mult)
                    st["v"] = v_t
                else:
                    v_t = st["v"]
                    sig_t = gv.tile([P, TH, CW], F32, tag="sig")
                    z_t = zpool.tile([P, TH, CW], F32, tag="z")
                    for i in range(TH):
                        cc = c1h[:, i:i + 1]
                        nc.scalar.activation(sig_t[:, i, :], pss[i], AF.Sigmoid, scale=cc)
                        nc.vector.tensor_mul(z_t[:, i, :], pss[i], sig_t[:, i, :])
                    nc.vector.tensor_mul(z_t, z_t, v_t)
                    nc.sync.dma_start(
                        out=z_rs[h][:, :, gi * CW:(gi + 1) * CW], in_=z_t
                    )
            return evac1

        def make_evac_plain(c_th, dst_r, use_h_offset):
            def evac(c, h, pss):
                o_t = zpool.tile([P, TH, CW], F32, tag="z")
                for i in range(TH):
                    cc = c_th[:, i:i + 1]
                    if i % 2 == 0:
                        nc.vector.tensor_scalar(o_t[:, i, :], pss[i], scalar1=cc,
                                                scalar2=None, op0=ALU.mult)
                    else:
                        nc.scalar.activation(o_t[:, i, :], pss[i], AF.Copy, scale=cc)
                row0 = h * TH if use_h_offset else 0
                nc.sync.dma_start(
                    out=dst_r[:, row0:row0 + TH, c * CW:(c + 1) * CW], in_=o_t
                )
            return evac

        # ---- pipelined layers: emission INTERLEAVED so quant(stage k+1)
        # overlaps mm(stage k) on every engine's instruction stream ----
        def stage_factory(L, h):
            def mk():
                ct = singles.tile([P, TH], F32, name=f"c{L}_{h}")
                if L == 1:
                    qT = qt_pool.tile([P, KT1, TH * P], BF16, tag="qt")
                    qg = quant_gen(x_p[:], KT1, h, qT, ct, c_gate, 1e-8)
                    mmf = lambda: mm_gen(wg_p, GCH, KT1, h, qT, make_evac1(ct))
                    return qg, mmf, GCH
                if L == 2:
                    qT = qt_pool.tile([P, KT2, TH * P], BF16, tag="qt")
                    qg = quant_gen(z_ds[h][:], KT2, 0, qT, ct, c_down, 1e-8)
                    mmf = lambda: mm_gen(wd_p, DCH, KT2, h, qT,
                                         make_evac_plain(ct, h_rs[h], False))
                    return qg, mmf, DCH
                qT = qt_pool.tile([P, KT1, TH * P], BF16, tag="qt")
                qg = quant_gen(h_ds[h][:], KT1, 0, qT, ct, c_out, 1e-8,
                               nw=True, nw_eps=1e-6)
                mmf = lambda: mm_gen(wo_p, DCH, KT1, h, qT,
                                     make_evac_plain(ct, out_r, True))
                return qg, mmf, DCH
            return mk

        stage_mks = [stage_factory(L, h) for L in (1, 2, 3) for h in range(NH)]
        qg0, mmf, nch = stage_mks[0]()
        for _ in qg0:
            pass
        for k in range(len(stage_mks)):
            mm = mmf()
            if k + 1 < len(stage_mks):
                qn, mmf, nch_next = stage_mks[k + 1]()
            else:
                qn = None
            step = max(1, nch // TH)
            ci = 0
            for _ in mm:
                ci += 1
                if qn is not None and ci % step == 0:
                    next(qn, None)
            if qn is not None:
                for _ in qn:
                    pass
            if k + 1 < len(stage_mks):
                nch = nch_next

    return nc


def prepare_inputs(condition, w_gate, w_down, norm_weight, w_out, cfg, n_cores=N_CORES):
    """Host-side: quantize+pack weights, shard tokens. Returns (in_maps, scales)."""
    TOK = condition.shape[0] * condition.shape[1]
    X = np.ascontiguousarray(condition.reshape(TOK, cfg.D).astype(np.float32, copy=False))

    Wg, sg = host_weight_quant(np.asarray(w_gate, dtype=np.float32))
    Wd, sd = host_weight_quant(np.asarray(w_down, dtype=np.float32))
    Wo, so = host_weight_quant(np.asarray(w_out, dtype=np.float32))

    # L1 chunk order interleaves v/gate so swiglu can fuse per chunk pair
    l1_cols = []
    for i in range(cfg.INTER // cfg.CW):
        l1_cols += [cfg.INTER + i * cfg.CW, i * cfg.CW]
    WG = pack_weight(Wg.T, l1_cols, cfg)
    WD = pack_weight(Wd.T, [i * cfg.CW for i in range(cfg.D // cfg.CW)], cfg)
    WO = pack_weight(Wo.T, [i * cfg.CW for i in range(cfg.D // cfg.CW)], cfg)

    nw = np.ascontiguousarray(np.asarray(norm_weight, dtype=np.float32).reshape(1, cfg.D))

    in_maps = []
    for i in range(n_cores):
        in_maps.append({
            "x": np.ascontiguousarray(X[i * cfg.T:(i + 1) * cfg.T]),
            "wg": WG, "wd": WD, "wo": WO, "nw": nw,
        })
    return in_maps, (sg, sd, so)


def run(condition, w_gate, w_down, norm_weight, w_out, cfg=None, trace=False, tmpdir=None):
    from concourse.bass_utils import run_bass_kernel_spmd
    if cfg is None:
        cfg = Cfg()
    in_maps, (sg, sd, so) = prepare_inputs(condition, w_gate, w_down, norm_weight, w_out, cfg)
    nc = build_nc(cfg, sg, sd, so)
    nc.finalize()
    res = run_bass_kernel_spmd(nc, in_maps, list(range(N_CORES)), trace=trace, tmpdir=tmpdir)
    outs = np.concatenate([np.asarray(res.results[i]["out"]) for i in range(N_CORES)], axis=0)
    B, S = condition.shape[0], condition.shape[1]
    Pfull = outs.reshape(B, S, cfg.D)
    H = cfg.D // 2
    return (Pfull[..., :H], Pfull[..., H:]), res


def kernel(condition, w_gate, w_down, norm_weight, w_out):
    (scale, shift), _ = run(condition, w_gate, w_down, norm_weight, w_out)
    return scale, shift
